# revision 33
# baseline (speedup 1.0000x reference)
"""Trainium2 Bass kernel for nn_AttnGreedySearch (attn greedy top-1 search).

Math restructure (exact in exact arithmetic):
  With A_t = W_k^t and c_t = b_k @ sum_{i<t} W_k^i (row form), the iterated
  corpus is ic_t = ic0 @ A_t + c_t where ic0 = X @ W_proj + b_proj.  Define
  the per-sample query column u~_j = A_{j+1} @ S_j with S_j = user + sum v_i
  (unnormalized running sum; positive scale + constant shift never change the
  argmax; softmax is monotonic so it is argmax-irrelevant).  Per iteration j:
      score'_j[s] = <ic0[s, :], u~_j>          (argmax-equivalent scores)
      g_j = ic0[argmax]                         (one-hot select, 16-dim)
      v_j = A_{j+1}^T g_j + c_{j+1}             (output row, exact)
      u~_{j+1} = W_k u~_j + M_j g_j + d_j,      M_j = A_{j+2} A_{j+1}^T,
                                                d_j = A_{j+2} c_{j+1}

Performance structure (fp16 end-to-end):
  - Host pre-casts X to fp16 and pre-transposes it into per-supertile blocks
    XT[st, d, (i,s)] with a ones row (bias folded into the contraction).
    Host also computes u~_0 = W_k @ user (tiny 16x16 transform) in both
    sample-major and feature-major layouts, and fills the j=0 output rows
    directly -- the device computes and stores only rows 1..5 in fp16.
  - P1: per item-tile matmul(lhsT=XT chunk [101,128] stationary, rhs=waug
    [101,16] moving) -> psum [128,16] lands ALREADY sample-major; one ACT
    copy per 128-sample tile writes ic0a fp16.
  - P2 processes pairs of 1024-sample groups: DVE runs prod, a one-level
    fp16 pair-fold over h then an 8-wide reduce (cheaper than a 16-wide 1x
    reduce), argmax, the 10-block half of the one-hot select and the item
    fold tree; GPSIMD multiplies the other 6 blocks reading the compact
    fp16 mask through a broadcast AP (no expansion); ACT expands the mask
    only for DVE's half, applies per-partition bias adds, and drains psum;
    PE runs the 128x128 block-diagonal fp16 recurrence.
  - Four pair-chains are software-pipelined; all DMA is batched into a few
    large transfers.
"""

import numpy as np

import concourse.bass as bass
import concourse.mybir as mybir
import concourse.tile as tile
from concourse import bacc
from concourse.bass_utils import run_bass_kernel_spmd
from concourse.masks import make_identity

F32 = mybir.dt.float32
F16 = mybir.dt.float16
SEARCH_NUM = 5
NCORES = 8
D = 100   # item feature dim
DA = D + 1
NSI = 10  # items per sample
H = 16    # projected dim
SH = NSI * H  # 160


def _host_constants(W_proj, b_proj, W_k, b_k):
    Wk = W_k.astype(np.float64)
    bk = b_k.astype(np.float64)
    A = [np.eye(H)]
    for _ in range(SEARCH_NUM + 1):
        A.append(A[-1] @ Wk)
    c = [np.zeros(H)]
    for _ in range(SEARCH_NUM + 1):
        c.append(c[-1] @ Wk + bk)

    def blkdiag8(m):
        out = np.zeros((128, 128))
        for t in range(8):
            out[t * H:(t + 1) * H, t * H:(t + 1) * H] = m
        return out.astype(np.float16)

    # blks packed [128, 10*128] fp16: wk, a0..a4, m0..m3
    blks = [blkdiag8(Wk.T)]
    for j in range(SEARCH_NUM):
        blks.append(blkdiag8(A[j + 1]))
    for j in range(SEARCH_NUM - 1):
        Mj = A[j + 2] @ A[j + 1].T
        blks.append(blkdiag8(Mj.T))
    blks = np.concatenate(blks, axis=1)  # [128, 1280] fp16

    # cv0..cv4, dv0..dv3 packed [128, 9] fp32
    cvdv = []
    for j in range(SEARCH_NUM):
        cvdv.append(np.tile(c[j + 1], 8).astype(np.float32)[:, None])
    for j in range(SEARCH_NUM - 1):
        dj = A[j + 2] @ c[j + 1]
        cvdv.append(np.tile(dj, 8).astype(np.float32)[:, None])
    cvdv = np.concatenate(cvdv, axis=1)  # [128, 9] fp32

    waug = np.zeros((DA, H), dtype=np.float16)
    waug[:D, :] = W_proj.astype(np.float16)
    waug[D, :] = b_proj.astype(np.float16)
    return {"blks": blks, "cvdv": cvdv, "waug": waug, "Wk": Wk}


def _v(t, off, dims, nparts=None):
    """View on tile/AP t: free dims `dims`, element offset `off` added.
    `nparts` overrides the partition count (step preserved)."""
    p = list(t.ap[0])
    if nparts is not None:
        p = [p[0], nparts]
    return bass.AP(tensor=t.tensor, offset=t.offset + off,
                   ap=[p] + [list(d) for d in dims])


def build_program(nc, B):
    assert B == 8192, "schedule is hardcoded for 16 supertiles / 4 pairs"
    NT = B // 128
    NST = B // 512
    NG = B // 1024
    mult = mybir.AluOpType.mult
    add = mybir.AluOpType.add
    iseq = mybir.AluOpType.is_equal

    # XT stored per-supertile: [NST, 101, 5248]; row r=(st,d) holds the 512
    # samples x 10 items block (i-major), padded 5120->5248 so the DRAM row
    # stride (41 x 256B pages) is coprime with the 16 DMA queues.
    XTW = NSI * 512 + 128
    xt_d = nc.dram_tensor("xt", [B // 512, 128, XTW], F16,
                          kind="ExternalInput").ap()
    ua0_d = nc.dram_tensor("ua0", [128, NG * 128], F16,
                           kind="ExternalInput").ap()
    ud0_d = nc.dram_tensor("ud0", [128, NG * 128], F16,
                           kind="ExternalInput").ap()
    waug_d = nc.dram_tensor("waug", [DA, H], F16, kind="ExternalInput").ap()
    blks_d = nc.dram_tensor("blks", [128, 1280], F16, kind="ExternalInput").ap()
    cvdv_d = nc.dram_tensor("cvdv", [128, 9], F32, kind="ExternalInput").ap()
    out_d = nc.dram_tensor("out", [B, SEARCH_NUM, H], F16,
                           kind="ExternalOutput").ap()

    with tile.TileContext(nc) as tc:
        with tc.tile_pool(name="singles", bufs=1) as singles, \
             tc.tile_pool(name="xst", bufs=6) as xst, \
             tc.tile_pool(name="scr", bufs=4) as scr, \
             tc.tile_pool(name="vop", bufs=1) as vop, \
             tc.tile_pool(name="ppr", bufs=4, space="PSUM") as ppr, \
             tc.tile_pool(name="pp2", bufs=2, space="PSUM") as pp2:

            # ---- persistent SBUF (waug first: P1 needs it; rest after the
            # first xt supertiles are queued) ----
            ident16 = singles.tile([128, 128], F16)
            waug_sb = singles.tile([DA, H], F16)
            nc.sync.dma_start(out=waug_sb, in_=waug_d)
            make_identity(nc, ident16)
            blks_sb = singles.tile([128, 1280], F16)
            cvdv_sb = singles.tile([128, 9], F32)

            def emit_warmup(n):
                # keep PE busy (and the HAM clock-gate open) while the first
                # supertiles stream in; results are never read
                for k in range(n):
                    wp = pp2.tile([128, 128], F32, name="warm", tag="p2f32")
                    nc.tensor.matmul(wp, ident16, ident16,
                                     start=True, stop=True)

            def blk_wk():
                return blks_sb[:, 0:128]

            def blk_a(j):
                return blks_sb[:, (1 + j) * 128:(2 + j) * 128]

            def blk_m(j):
                return blks_sb[:, (6 + j) * 128:(7 + j) * 128]

            def cv(j):
                return cvdv_sb[:, j:j + 1]

            def dv(j):
                return cvdv_sb[:, 5 + j:6 + j]

            ic0a = singles.tile([128, NT * SH], F16)   # (t,i,h) per tile
            ua = singles.tile([128, NG * 128], F16)     # u~ sample-major
            ud = singles.tile([128, NG * 128], F16)     # u~ feature-major

            def emit_uaud():
                nc.sync.dma_start(out=ua, in_=ua0_d)
                nc.sync.dma_start(out=ud, in_=ud0_d)

            def emit_blks():
                nc.sync.dma_start(out=blks_sb, in_=blks_d)
                nc.sync.dma_start(out=cvdv_sb, in_=cvdv_d)

            # ---- P1 for one super-tile of 512 samples ----
            # DMA dispatch and the matmuls are emitted separately: per-engine
            # execution is strictly in emission order, so a P1 matmul emitted
            # before a pair-iter's transposes would stall PE on the DMA.
            xt_tiles = {}

            def emit_p1_dma(st):
                xt_sb = xst.tile([128, NSI * 512], F16, name="xt_sb")
                src = bass.AP(tensor=xt_d.tensor,
                              offset=xt_d.offset + st * 128 * XTW,
                              ap=[[XTW, 128], [1, NSI * 512]])
                nc.sync.dma_start(out=xt_sb, in_=src)
                xt_tiles[st] = xt_sb

            def emit_p1_mm(st, drain=None):
                xt_sb = xt_tiles.pop(st)
                eng = drain if drain is not None else nc.scalar
                for a2 in range(2):
                    pc = ppr.tile([128, 2 * SH], F32, name="pc", tag="pc")
                    for a in (0, 1):
                        s0 = (2 * a2 + a) * 128
                        for i in range(NSI):
                            nc.tensor.matmul(
                                pc[:, a * SH + i * H:a * SH + (i + 1) * H],
                                xt_sb[:DA, i * 512 + s0:i * 512 + s0 + 128],
                                waug_sb, start=True, stop=True)
                    c_ = st * 4 + 2 * a2
                    if eng is nc.vector:
                        nc.vector.tensor_copy(
                            ic0a[:, c_ * SH:(c_ + 2) * SH], pc)
                    else:
                        eng.copy(ic0a[:, c_ * SH:(c_ + 2) * SH], pc)

            # ---- P2: one iteration for one PAIR of groups (2q, 2q+1),
            # split into phase A (score path, pure DVE) and phase B
            # (select + fold + recurrence) so the scheduler can interleave
            # phases of different pairs at sub-iteration granularity ----
            ab_state = {}

            def emit_A(q, j, half=None):
                # half=0/1 processes a single 1024-sample group (8 tiles) --
                # used on j=0 so a group starts after only 2 supertiles land
                h0 = 0 if half is None else half
                nt = 16 if half is None else 8
                base = q * 16 * SH + h0 * 8 * SH
                p0 = q * 256 + h0 * 128

                prod = scr.tile([128, 16, NSI, H], F16, name="prod",
                                tag="prod")
                nc.vector.tensor_tensor(
                    out=_v(prod, 0, [[SH, nt], [H, NSI], [1, H]]),
                    in0=_v(ic0a, base, [[SH, nt], [H, NSI], [1, H]]),
                    in1=_v(ua, p0, [[H, nt], [0, NSI], [1, H]]), op=mult)
                # score path: three fp16 pair-folds over h (16->8->4->2),
                # then a final fp32 pair-add
                sf1 = scr.tile([128, 16, NSI, 8], F16, name="sf1", tag="sf1")
                nc.vector.tensor_tensor(
                    out=_v(sf1, 0, [[NSI * 8, nt], [8, NSI], [1, 8]]),
                    in0=_v(prod, 0, [[SH, nt], [H, NSI], [1, 8]]),
                    in1=_v(prod, 8, [[SH, nt], [H, NSI], [1, 8]]), op=add)
                sf2 = scr.tile([128, 16, NSI, 4], F16, name="sf2", tag="sf2")
                nc.vector.tensor_tensor(
                    out=_v(sf2, 0, [[NSI * 4, nt], [4, NSI], [1, 4]]),
                    in0=_v(sf1, 0, [[NSI * 8, nt], [8, NSI], [1, 4]]),
                    in1=_v(sf1, 4, [[NSI * 8, nt], [8, NSI], [1, 4]]), op=add)
                sf3 = scr.tile([128, 16, NSI, 2], F16, name="sf3", tag="sf3")
                nc.vector.tensor_tensor(
                    out=_v(sf3, 0, [[NSI * 2, nt], [2, NSI], [1, 2]]),
                    in0=_v(sf2, 0, [[NSI * 4, nt], [4, NSI], [1, 2]]),
                    in1=_v(sf2, 2, [[NSI * 4, nt], [4, NSI], [1, 2]]), op=add)
                scores = scr.tile([128, 16, NSI], F32, name="scores",
                                  tag="scores")
                nc.vector.tensor_tensor(
                    out=_v(scores, 0, [[NSI, nt], [1, NSI]]),
                    in0=_v(sf3, 0, [[NSI * 2, nt], [2, NSI]]),
                    in1=_v(sf3, 1, [[NSI * 2, nt], [2, NSI]]), op=add)
                mx = scr.tile([128, 16], F32, name="mx", tag="mx")
                nc.vector.reduce_max(out=_v(mx, 0, [[1, nt]]),
                                     in_=_v(scores, 0, [[NSI, nt], [1, NSI]]),
                                     axis=mybir.AxisListType.X)
                mask = scr.tile([128, 16, NSI], F32, name="mask", tag="mask")
                nc.vector.tensor_tensor(
                    out=_v(mask, 0, [[NSI, nt], [1, NSI]]),
                    in0=_v(scores, 0, [[NSI, nt], [1, NSI]]),
                    in1=_v(mx, 0, [[1, nt], [0, NSI]]), op=iseq)
                ab_state[(q, half)] = mask

            def emit_B(q, j, half=None):
                h0 = 0 if half is None else half
                ng = 2 if half is None else 1
                nt = 8 * ng
                nb = 4 * ng
                base = q * 16 * SH + h0 * 8 * SH
                p0 = q * 256 + h0 * 128
                mask = ab_state.pop((q, half))
                # ACT expands the mask (broadcast reads run at 1x); DVE then
                # applies it to the two block-halves in two 2x ops.  GPSIMD
                # is kept out of P2 -- it steals DVE's SBUF port (measured
                # net loss).
                h8 = [[SH, nb], [H, NSI], [1, H]]
                maskE = scr.tile([128, 16, NSI, H], F16, name="maskE",
                                 tag="maskE")
                sel = scr.tile([128, 16, NSI, H], F16, name="sel", tag="sel")
                nc.scalar.copy(
                    _v(maskE, 0, h8),
                    _v(mask, 0, [[NSI, nb], [1, NSI], [0, H]]))
                nc.scalar.copy(
                    _v(maskE, nb * SH, h8),
                    _v(mask, nb * NSI, [[NSI, nb], [1, NSI], [0, H]]))
                nc.vector.tensor_tensor(
                    out=_v(sel, 0, h8),
                    in0=_v(ic0a, base, h8),
                    in1=_v(maskE, 0, h8), op=mult)
                nc.vector.tensor_tensor(
                    out=_v(sel, nb * SH, h8),
                    in0=_v(ic0a, base + nb * SH, h8),
                    in1=_v(maskE, nb * SH, h8), op=mult)
                # pairwise fold over items: 10 -> 5 -> (4->2->1) + leftover
                f1 = scr.tile([128, 16, 5, H], F16, name="f1", tag="f1")
                nc.vector.tensor_tensor(
                    out=_v(f1, 0, [[5 * H, nt], [H, 5], [1, H]]),
                    in0=_v(sel, 0, [[SH, nt], [H, 5], [1, H]]),
                    in1=_v(sel, 5 * H, [[SH, nt], [H, 5], [1, H]]), op=add)
                f2 = scr.tile([128, 16, 2, H], F16, name="f2", tag="f2")
                nc.vector.tensor_tensor(
                    out=_v(f2, 0, [[2 * H, nt], [H, 2], [1, H]]),
                    in0=_v(f1, 0, [[5 * H, nt], [H, 2], [1, H]]),
                    in1=_v(f1, 2 * H, [[5 * H, nt], [H, 2], [1, H]]), op=add)
                f3 = scr.tile([128, 16, H], F16, name="f3", tag="f3")
                nc.vector.tensor_tensor(
                    out=_v(f3, 0, [[H, nt], [1, H]]),
                    in0=_v(f2, 0, [[2 * H, nt], [1, H]]),
                    in1=_v(f2, H, [[2 * H, nt], [1, H]]), op=add)
                ga = scr.tile([128, 16, H], F16, name="ga", tag="ga")
                nc.vector.tensor_tensor(
                    out=_v(ga, 0, [[H, nt], [1, H]]),
                    in0=_v(f3, 0, [[H, nt], [1, H]]),
                    in1=_v(f1, 4 * H, [[5 * H, nt], [1, H]]),
                    op=add)

                tpg = pp2.tile([128, 256], F16, name="tpg", tag="p2f16")
                for g in range(ng):
                    nc.tensor.transpose(tpg[:, g * 128:(g + 1) * 128],
                                        _v(ga, g * 128, [[1, 128]]), ident16)
                gd16 = scr.tile([128, 256], F16, name="gd16", tag="gd")
                nc.scalar.copy(gd16[:, :128 * ng], tpg[:, :128 * ng])
                # u~ recurrence first: cross-iteration critical path
                if j < SEARCH_NUM - 1:
                    up = pp2.tile([128, 256], F32, name="upj", tag="p2f32")
                    for g in range(ng):
                        c0, c1 = g * 128, (g + 1) * 128
                        nc.tensor.matmul(up[:, c0:c1], blk_wk(),
                                         ud[:, p0 + c0:p0 + c1], start=True,
                                         stop=False)
                        nc.tensor.matmul(up[:, c0:c1], blk_m(j),
                                         gd16[:, c0:c1],
                                         start=False, stop=True)
                    nc.scalar.add(ud[:, p0:p0 + 128 * ng],
                                  up[:, :128 * ng], dv(j))
                    tpu = pp2.tile([128, 256], F16, name="tpu", tag="p2f16")
                    for g in range(ng):
                        c0, c1 = g * 128, (g + 1) * 128
                        nc.tensor.transpose(tpu[:, c0:c1],
                                            ud[:, p0 + c0:p0 + c1], ident16)
                    nc.scalar.copy(ua[:, p0:p0 + 128 * ng], tpu[:, :128 * ng])
                vp = pp2.tile([128, 256], F32, name="vp", tag="p2f32")
                for g in range(ng):
                    c0, c1 = g * 128, (g + 1) * 128
                    nc.tensor.matmul(vp[:, c0:c1], blk_a(j), gd16[:, c0:c1],
                                     start=True, stop=True)
                vtmp = scr.tile([128, 256], F16, name="vtmp", tag="vtmp")
                nc.scalar.add(vtmp[:, :128 * ng], vp[:, :128 * ng], cv(j))
                tpv = pp2.tile([128, 256], F16, name="tpv", tag="p2f16")
                for g in range(ng):
                    c0, c1 = g * 128, (g + 1) * 128
                    nc.tensor.transpose(tpv[:, c0:c1], vtmp[:, c0:c1],
                                        ident16)
                vout = vouts[q]
                nc.scalar.copy(
                    _v(vout, h0 * 640 + j * H, [[640, ng], [80, 8], [1, H]]),
                    tpv[:, :128 * ng])

            def emit_p3(g):
                vout = vouts[g // 2]
                dst = bass.AP(
                    tensor=out_d.tensor,
                    offset=out_d.offset + g * 1024 * 80,
                    ap=[[80, 128], [128 * 80, 8], [1, 80]])
                nc.sync.dma_start(out=dst,
                                  in_=_v(vout, (g % 2) * 640,
                                         [[80, 8], [1, 80]]))

            vouts = [vop.tile([128, 2 * 8 * 80], F16, name=f"vout{k}")
                     for k in range(4)]

            # ---- main schedule: round-robin software-pipelined chains ----
            # Emission order IS the per-engine schedule (strict in-order
            # execution), so: xt DMA dispatches go out as early as the 6
            # rotating buffers allow, P1 matmuls are emitted only where the
            # data has landed by that point of the schedule, the four
            # pair-chains are interleaved, and pairs 0/1 hold back their
            # final iterations as end-game filler while chains 2/3 drain.
            def emit_p1(st, drain=None):
                emit_p1_dma(st)
                emit_p1_mm(st, drain)

            emit_p1(0)
            emit_p1(1)
            emit_uaud()
            emit_blks()
            plan = [
                ("A", 0, 0, 0), ("p", 2), ("B", 0, 0, 0), ("p", 3),
                ("A", 0, 0, 1), ("B", 0, 0, 1), ("p", 4), ("p", 5),
                ("A", 1, 0, 0), ("B", 1, 0, 0), ("p", 6), ("p", 7),
                ("A", 0, 1, None), ("A", 1, 0, 1), ("B", 0, 1, None),
                ("B", 1, 0, 1), ("p", 8), ("A", 1, 1, None), ("p", 9),
                ("A", 0, 2, None), ("B", 1, 1, None), ("A", 2, 0, 0),
                ("B", 0, 2, None), ("B", 2, 0, 0), ("p", 10), ("p", 11),
                ("A", 1, 2, None), ("A", 0, 3, None), ("B", 1, 2, None),
                ("B", 0, 3, None), ("A", 2, 0, 1), ("B", 2, 0, 1),
                ("p", 12), ("p", 13), ("A", 2, 1, None), ("A", 1, 3, None),
                ("B", 2, 1, None), ("B", 1, 3, None), ("A", 3, 0, 0),
                ("B", 3, 0, 0), ("p", 14), ("p", 15), ("A", 0, 4, None),
                ("B", 0, 4, None), ("A", 3, 0, 1), ("B", 3, 0, 1),
                ("A", 2, 2, None), ("B", 2, 2, None), ("A", 1, 4, None),
                ("B", 1, 4, None), ("A", 3, 1, None), ("B", 3, 1, None),
                ("A", 2, 3, None), ("B", 2, 3, None), ("A", 3, 2, None),
                ("B", 3, 2, None), ("A", 2, 4, None), ("B", 2, 4, None),
                ("A", 3, 3, None), ("B", 3, 3, None), ("A", 3, 4, None),
                ("B", 3, 4, None),
            ]
            for step in plan:
                if step[0] == "p":
                    emit_p1(step[1])
                elif step[0] == "A":
                    emit_A(step[1], step[2], step[3])
                else:
                    _, q, j, half = step
                    emit_B(q, j, half)
                    if j == SEARCH_NUM - 1:
                        emit_p3(2 * q)
                        emit_p3(2 * q + 1)


def _in_maps(inputs, B_core):
    cst = _host_constants(inputs["W_proj"], inputs["b_proj"],
                          inputs["W_k"], inputs["b_k"])
    x = np.asarray(inputs["item_corpus"], dtype=np.float32)
    u = np.ascontiguousarray(inputs["user_intent"], dtype=np.float32)
    B = x.shape[0]
    NST = B_core // 512
    NG = B_core // 1024
    XTW = NSI * 512 + 128
    # [nst_total, 100, 10, 512]: block (st, d, i, s)
    xb = (x.astype(np.float16)
          .reshape(B // 512, 512, NSI, D)
          .transpose(0, 3, 2, 1))
    # u~_0 = W_k @ user (column form) == user @ W_k.T (row form)
    U0 = (u.astype(np.float64) @ cst["Wk"].T).astype(np.float16)
    maps = []
    for core in range(NCORES):
        xt = np.zeros((NST, 128, XTW), dtype=np.float16)
        xt[:, :D, :NSI * 512] = xb[core * NST:(core + 1) * NST].reshape(
            NST, D, NSI * 512)
        xt[:, D, :NSI * 512] = 1.0
        lo, hi = core * B_core, (core + 1) * B_core
        u0r = U0[lo:hi].reshape(NG, 8, 128, H)
        ua0 = np.ascontiguousarray(
            u0r.transpose(2, 0, 1, 3).reshape(128, NG * 128))
        ud0 = np.ascontiguousarray(
            u0r.transpose(1, 3, 0, 2).reshape(128, NG * 128))
        m = {"xt": xt, "ua0": ua0, "ud0": ud0,
             "waug": cst["waug"], "blks": cst["blks"], "cvdv": cst["cvdv"]}
        maps.append(m)
    return maps


def _assemble(inputs, results):
    u = np.asarray(inputs["user_intent"], dtype=np.float32)
    bs = u.shape[0]
    dev = np.concatenate([np.asarray(r["out"]) for r in results], axis=0)
    out = np.empty((bs, SEARCH_NUM + 1, H), dtype=np.float32)
    out[:, 0, :] = u
    out[:, 1:, :] = dev.astype(np.float32)
    return out


_COMPILED = {}


def _get_nc(B_core):
    if B_core not in _COMPILED:
        nc = bacc.Bacc("TRN2", target_bir_lowering=False, debug=False,
                       num_devices=NCORES)
        build_program(nc, B_core)
        nc.compile()
        _COMPILED[B_core] = nc
    return _COMPILED[B_core]


def kernel(**inputs) -> np.ndarray:
    bs = inputs["user_intent"].shape[0]
    assert bs % NCORES == 0
    B_core = bs // NCORES
    nc = _get_nc(B_core)
    res = run_bass_kernel_spmd(nc, _in_maps(inputs, B_core),
                               core_ids=list(range(NCORES)))
    return _assemble(inputs, res.results)


# revision 35
# speedup vs baseline: 1.0711x; 1.0711x over previous
"""Trainium2 Bass kernel for nn_AttnGreedySearch (attn greedy top-1 search).

Math restructure (exact in exact arithmetic):
  With A_t = W_k^t and c_t = b_k @ sum_{i<t} W_k^i (row form), the iterated
  corpus is ic_t = ic0 @ A_t + c_t where ic0 = X @ W_proj + b_proj.  Define
  the per-sample query column u~_j = A_{j+1} @ S_j with S_j = user + sum v_i
  (unnormalized running sum; positive scale + constant shift never change the
  argmax; softmax is monotonic so it is argmax-irrelevant).  Per iteration j:
      score'_j[s] = <ic0[s, :], u~_j>          (argmax-equivalent scores)
      g_j = ic0[argmax]                         (one-hot select, 16-dim)
      v_j = A_{j+1}^T g_j + c_{j+1}             (output row, exact)
      u~_{j+1} = W_k u~_j + M_j g_j + d_j,      M_j = A_{j+2} A_{j+1}^T,
                                                d_j = A_{j+2} c_{j+1}

Performance structure (fp16 end-to-end):
  - Host pre-casts X to fp16 and pre-transposes it into per-supertile blocks
    XT[st, d, (i,s)] with a ones row (bias folded into the contraction).
    Host also computes u~_0 = W_k @ user (tiny 16x16 transform) in both
    sample-major and feature-major layouts, and fills the j=0 output rows
    directly -- the device computes and stores only rows 1..5 in fp16.
  - P1: per item-tile matmul(lhsT=XT chunk [101,128] stationary, rhs=waug
    [101,16] moving) -> psum [128,16] lands ALREADY sample-major; one ACT
    copy per 128-sample tile writes ic0a fp16.
  - P2 processes pairs of 1024-sample groups: DVE runs prod, a one-level
    fp16 pair-fold over h then an 8-wide reduce (cheaper than a 16-wide 1x
    reduce), argmax, the 10-block half of the one-hot select and the item
    fold tree; GPSIMD multiplies the other 6 blocks reading the compact
    fp16 mask through a broadcast AP (no expansion); ACT expands the mask
    only for DVE's half, applies per-partition bias adds, and drains psum;
    PE runs the 128x128 block-diagonal fp16 recurrence.
  - Four pair-chains are software-pipelined; all DMA is batched into a few
    large transfers.
"""

import numpy as np

import concourse.bass as bass
import concourse.mybir as mybir
import concourse.tile as tile
from concourse import bacc
from concourse.bass_utils import run_bass_kernel_spmd
from concourse.masks import make_identity

F32 = mybir.dt.float32
F16 = mybir.dt.float16
SEARCH_NUM = 5
NCORES = 8
D = 100   # item feature dim
DA = D + 1
NSI = 10  # items per sample
H = 16    # projected dim
SH = NSI * H  # 160


def _host_constants(W_proj, b_proj, W_k, b_k):
    Wk = W_k.astype(np.float64)
    bk = b_k.astype(np.float64)
    A = [np.eye(H)]
    for _ in range(SEARCH_NUM + 1):
        A.append(A[-1] @ Wk)
    c = [np.zeros(H)]
    for _ in range(SEARCH_NUM + 1):
        c.append(c[-1] @ Wk + bk)

    def blkdiag8(m):
        out = np.zeros((128, 128))
        for t in range(8):
            out[t * H:(t + 1) * H, t * H:(t + 1) * H] = m
        return out.astype(np.float16)

    # blks packed [128, 10*128] fp16: wk, a0..a4, m0..m3
    blks = [blkdiag8(Wk.T)]
    for j in range(SEARCH_NUM):
        blks.append(blkdiag8(A[j + 1]))
    for j in range(SEARCH_NUM - 1):
        Mj = A[j + 2] @ A[j + 1].T
        blks.append(blkdiag8(Mj.T))
    blks = np.concatenate(blks, axis=1)  # [128, 1280] fp16

    # cv0..cv4, dv0..dv3 packed [128, 9] fp32
    cvdv = []
    for j in range(SEARCH_NUM):
        cvdv.append(np.tile(c[j + 1], 8).astype(np.float32)[:, None])
    for j in range(SEARCH_NUM - 1):
        dj = A[j + 2] @ c[j + 1]
        cvdv.append(np.tile(dj, 8).astype(np.float32)[:, None])
    cvdv = np.concatenate(cvdv, axis=1)  # [128, 9] fp32

    waug = np.zeros((DA, H), dtype=np.float16)
    waug[:D, :] = W_proj.astype(np.float16)
    waug[D, :] = b_proj.astype(np.float16)
    return {"blks": blks, "cvdv": cvdv, "waug": waug, "Wk": Wk}


def _v(t, off, dims, nparts=None):
    """View on tile/AP t: free dims `dims`, element offset `off` added.
    `nparts` overrides the partition count (step preserved)."""
    p = list(t.ap[0])
    if nparts is not None:
        p = [p[0], nparts]
    return bass.AP(tensor=t.tensor, offset=t.offset + off,
                   ap=[p] + [list(d) for d in dims])


def build_program(nc, B):
    assert B == 8192, "schedule is hardcoded for 16 supertiles / 4 pairs"
    NT = B // 128
    NST = B // 512
    NG = B // 1024
    mult = mybir.AluOpType.mult
    add = mybir.AluOpType.add
    iseq = mybir.AluOpType.is_equal

    # XT stored per-supertile: [NST, 101, 5248]; row r=(st,d) holds the 512
    # samples x 10 items block (i-major), padded 5120->5248 so the DRAM row
    # stride (41 x 256B pages) is coprime with the 16 DMA queues.
    XTW = NSI * 512 + 128
    xt_d = nc.dram_tensor("xt", [B // 512, 128, XTW], F16,
                          kind="ExternalInput").ap()
    ua0_d = nc.dram_tensor("ua0", [128, NG * 128], F16,
                           kind="ExternalInput").ap()
    ud0_d = nc.dram_tensor("ud0", [128, NG * 128], F16,
                           kind="ExternalInput").ap()
    waug_d = nc.dram_tensor("waug", [DA, H], F16, kind="ExternalInput").ap()
    blks_d = nc.dram_tensor("blks", [128, 1280], F16, kind="ExternalInput").ap()
    cvdv_d = nc.dram_tensor("cvdv", [128, 9], F32, kind="ExternalInput").ap()
    out_d = nc.dram_tensor("out", [B, SEARCH_NUM, H], F16,
                           kind="ExternalOutput").ap()

    with tile.TileContext(nc) as tc:
        with tc.tile_pool(name="singles", bufs=1) as singles, \
             tc.tile_pool(name="xst", bufs=6) as xst, \
             tc.tile_pool(name="scr", bufs=4) as scr, \
             tc.tile_pool(name="vop", bufs=1) as vop, \
             tc.tile_pool(name="ppr", bufs=4, space="PSUM") as ppr, \
             tc.tile_pool(name="pp2", bufs=2, space="PSUM") as pp2:

            # ---- persistent SBUF (waug first: P1 needs it; rest after the
            # first xt supertiles are queued) ----
            ident16 = singles.tile([128, 128], F16)
            waug_sb = singles.tile([DA, H], F16)
            nc.sync.dma_start(out=waug_sb, in_=waug_d)
            make_identity(nc, ident16)
            blks_sb = singles.tile([128, 1280], F16)
            cvdv_sb = singles.tile([128, 9], F32)

            def emit_warmup(n):
                # keep PE busy (and the HAM clock-gate open) while the first
                # supertiles stream in; results are never read
                for k in range(n):
                    wp = pp2.tile([128, 128], F32, name="warm", tag="p2f32")
                    nc.tensor.matmul(wp, ident16, ident16,
                                     start=True, stop=True)

            def blk_wk():
                return blks_sb[:, 0:128]

            def blk_a(j):
                return blks_sb[:, (1 + j) * 128:(2 + j) * 128]

            def blk_m(j):
                return blks_sb[:, (6 + j) * 128:(7 + j) * 128]

            def cv(j):
                return cvdv_sb[:, j:j + 1]

            def dv(j):
                return cvdv_sb[:, 5 + j:6 + j]

            ic0a = singles.tile([128, NT * SH], F16)   # (t,i,h) per tile
            ua = singles.tile([128, NG * 128], F16)     # u~ sample-major
            ud = singles.tile([128, NG * 128], F16)     # u~ feature-major

            def emit_uaud():
                nc.sync.dma_start(out=ua, in_=ua0_d)
                nc.sync.dma_start(out=ud, in_=ud0_d)

            def emit_blks():
                nc.sync.dma_start(out=blks_sb, in_=blks_d)
                nc.sync.dma_start(out=cvdv_sb, in_=cvdv_d)

            # ---- P1 for one super-tile of 512 samples ----
            # DMA dispatch and the matmuls are emitted separately: per-engine
            # execution is strictly in emission order, so a P1 matmul emitted
            # before a pair-iter's transposes would stall PE on the DMA.
            xt_tiles = {}

            def emit_p1_dma(st):
                xt_sb = xst.tile([128, NSI * 512], F16, name="xt_sb")
                src = bass.AP(tensor=xt_d.tensor,
                              offset=xt_d.offset + st * 128 * XTW,
                              ap=[[XTW, 128], [1, NSI * 512]])
                nc.sync.dma_start(out=xt_sb, in_=src)
                xt_tiles[st] = xt_sb

            def emit_p1_mm(st, drain=None):
                xt_sb = xt_tiles.pop(st)
                eng = drain if drain is not None else nc.scalar
                for a2 in range(2):
                    pc = ppr.tile([128, 2 * SH], F32, name="pc", tag="pc")
                    for a in (0, 1):
                        s0 = (2 * a2 + a) * 128
                        for i in range(NSI):
                            nc.tensor.matmul(
                                pc[:, a * SH + i * H:a * SH + (i + 1) * H],
                                xt_sb[:DA, i * 512 + s0:i * 512 + s0 + 128],
                                waug_sb, start=True, stop=True)
                    c_ = st * 4 + 2 * a2
                    if eng is nc.vector:
                        nc.vector.tensor_copy(
                            ic0a[:, c_ * SH:(c_ + 2) * SH], pc)
                    else:
                        eng.copy(ic0a[:, c_ * SH:(c_ + 2) * SH], pc)

            # ---- P2: one iteration for one PAIR of groups (2q, 2q+1),
            # split into phase A (score path, pure DVE) and phase B
            # (select + fold + recurrence) so the scheduler can interleave
            # phases of different pairs at sub-iteration granularity ----
            ab_state = {}

            def emit_A(q, j, half=None):
                # half=0/1 processes a single 1024-sample group (8 tiles) --
                # used on j=0 so a group starts after only 2 supertiles land
                h0 = 0 if half is None else half
                nt = 16 if half is None else 8
                base = q * 16 * SH + h0 * 8 * SH
                p0 = q * 256 + h0 * 128

                prod = scr.tile([128, 16, NSI, H], F16, name="prod",
                                tag="prod")
                nc.vector.tensor_tensor(
                    out=_v(prod, 0, [[SH, nt], [H, NSI], [1, H]]),
                    in0=_v(ic0a, base, [[SH, nt], [H, NSI], [1, H]]),
                    in1=_v(ua, p0, [[H, nt], [0, NSI], [1, H]]), op=mult)
                # score path: three fp16 pair-folds over h (16->8->4->2),
                # then a final fp32 pair-add
                sf1 = scr.tile([128, 16, NSI, 8], F16, name="sf1", tag="sf1")
                nc.vector.tensor_tensor(
                    out=_v(sf1, 0, [[NSI * 8, nt], [8, NSI], [1, 8]]),
                    in0=_v(prod, 0, [[SH, nt], [H, NSI], [1, 8]]),
                    in1=_v(prod, 8, [[SH, nt], [H, NSI], [1, 8]]), op=add)
                sf2 = scr.tile([128, 16, NSI, 4], F16, name="sf2", tag="sf2")
                nc.vector.tensor_tensor(
                    out=_v(sf2, 0, [[NSI * 4, nt], [4, NSI], [1, 4]]),
                    in0=_v(sf1, 0, [[NSI * 8, nt], [8, NSI], [1, 4]]),
                    in1=_v(sf1, 4, [[NSI * 8, nt], [8, NSI], [1, 4]]), op=add)
                scores = scr.tile([128, 16, NSI], F32, name="scores",
                                  tag="scores")
                nc.vector.reduce_sum(
                    out=_v(scores, 0, [[NSI, nt], [1, NSI]]),
                    in_=_v(sf2, 0, [[NSI * 4, nt], [4, NSI], [1, 4]]),
                    axis=mybir.AxisListType.X)
                mx = scr.tile([128, 16], F32, name="mx", tag="mx")
                nc.vector.reduce_max(out=_v(mx, 0, [[1, nt]]),
                                     in_=_v(scores, 0, [[NSI, nt], [1, NSI]]),
                                     axis=mybir.AxisListType.X)
                mask = scr.tile([128, 16, NSI], F32, name="mask", tag="mask")
                nc.vector.tensor_tensor(
                    out=_v(mask, 0, [[NSI, nt], [1, NSI]]),
                    in0=_v(scores, 0, [[NSI, nt], [1, NSI]]),
                    in1=_v(mx, 0, [[1, nt], [0, NSI]]), op=iseq)
                ab_state[(q, half)] = mask

            def emit_B(q, j, half=None):
                h0 = 0 if half is None else half
                ng = 2 if half is None else 1
                nt = 8 * ng
                nb = 4 * ng
                base = q * 16 * SH + h0 * 8 * SH
                p0 = q * 256 + h0 * 128
                mask = ab_state.pop((q, half))
                # ACT expands the mask (broadcast reads run at 1x); DVE then
                # applies it to the two block-halves in two 2x ops.  GPSIMD
                # is kept out of P2 -- it steals DVE's SBUF port (measured
                # net loss).
                h8 = [[SH, nb], [H, NSI], [1, H]]
                maskE = scr.tile([128, 16, NSI, H], F16, name="maskE",
                                 tag="maskE")
                sel = scr.tile([128, 16, NSI, H], F16, name="sel", tag="sel")
                nc.scalar.copy(
                    _v(maskE, 0, h8),
                    _v(mask, 0, [[NSI, nb], [1, NSI], [0, H]]))
                nc.scalar.copy(
                    _v(maskE, nb * SH, h8),
                    _v(mask, nb * NSI, [[NSI, nb], [1, NSI], [0, H]]))
                nc.vector.tensor_tensor(
                    out=_v(sel, 0, h8),
                    in0=_v(ic0a, base, h8),
                    in1=_v(maskE, 0, h8), op=mult)
                nc.vector.tensor_tensor(
                    out=_v(sel, nb * SH, h8),
                    in0=_v(ic0a, base + nb * SH, h8),
                    in1=_v(maskE, nb * SH, h8), op=mult)
                # pairwise fold over items: 10 -> 5 -> (4->2->1) + leftover
                f1 = scr.tile([128, 16, 5, H], F16, name="f1", tag="f1")
                nc.vector.tensor_tensor(
                    out=_v(f1, 0, [[5 * H, nt], [H, 5], [1, H]]),
                    in0=_v(sel, 0, [[SH, nt], [H, 5], [1, H]]),
                    in1=_v(sel, 5 * H, [[SH, nt], [H, 5], [1, H]]), op=add)
                f2 = scr.tile([128, 16, 2, H], F16, name="f2", tag="f2")
                nc.vector.tensor_tensor(
                    out=_v(f2, 0, [[2 * H, nt], [H, 2], [1, H]]),
                    in0=_v(f1, 0, [[5 * H, nt], [H, 2], [1, H]]),
                    in1=_v(f1, 2 * H, [[5 * H, nt], [H, 2], [1, H]]), op=add)
                f3 = scr.tile([128, 16, H], F16, name="f3", tag="f3")
                nc.vector.tensor_tensor(
                    out=_v(f3, 0, [[H, nt], [1, H]]),
                    in0=_v(f2, 0, [[2 * H, nt], [1, H]]),
                    in1=_v(f2, H, [[2 * H, nt], [1, H]]), op=add)
                ga = scr.tile([128, 16, H], F16, name="ga", tag="ga")
                nc.vector.tensor_tensor(
                    out=_v(ga, 0, [[H, nt], [1, H]]),
                    in0=_v(f3, 0, [[H, nt], [1, H]]),
                    in1=_v(f1, 4 * H, [[5 * H, nt], [1, H]]),
                    op=add)

                tpg = pp2.tile([128, 256], F16, name="tpg", tag="p2f16")
                for g in range(ng):
                    nc.tensor.transpose(tpg[:, g * 128:(g + 1) * 128],
                                        _v(ga, g * 128, [[1, 128]]), ident16)
                gd16 = scr.tile([128, 256], F16, name="gd16", tag="gd")
                nc.scalar.copy(gd16[:, :128 * ng], tpg[:, :128 * ng])
                # u~ recurrence first: cross-iteration critical path
                if j < SEARCH_NUM - 1:
                    up = pp2.tile([128, 256], F32, name="upj", tag="p2f32")
                    for g in range(ng):
                        c0, c1 = g * 128, (g + 1) * 128
                        nc.tensor.matmul(up[:, c0:c1], blk_wk(),
                                         ud[:, p0 + c0:p0 + c1], start=True,
                                         stop=False)
                        nc.tensor.matmul(up[:, c0:c1], blk_m(j),
                                         gd16[:, c0:c1],
                                         start=False, stop=True)
                    nc.scalar.add(ud[:, p0:p0 + 128 * ng],
                                  up[:, :128 * ng], dv(j))
                    tpu = pp2.tile([128, 256], F16, name="tpu", tag="p2f16")
                    for g in range(ng):
                        c0, c1 = g * 128, (g + 1) * 128
                        nc.tensor.transpose(tpu[:, c0:c1],
                                            ud[:, p0 + c0:p0 + c1], ident16)
                    nc.scalar.copy(ua[:, p0:p0 + 128 * ng], tpu[:, :128 * ng])
                vp = pp2.tile([128, 256], F32, name="vp", tag="p2f32")
                for g in range(ng):
                    c0, c1 = g * 128, (g + 1) * 128
                    nc.tensor.matmul(vp[:, c0:c1], blk_a(j), gd16[:, c0:c1],
                                     start=True, stop=True)
                vtmp = scr.tile([128, 256], F16, name="vtmp", tag="vtmp")
                nc.scalar.add(vtmp[:, :128 * ng], vp[:, :128 * ng], cv(j))
                tpv = pp2.tile([128, 256], F16, name="tpv", tag="p2f16")
                for g in range(ng):
                    c0, c1 = g * 128, (g + 1) * 128
                    nc.tensor.transpose(tpv[:, c0:c1], vtmp[:, c0:c1],
                                        ident16)
                vout = vouts[q]
                nc.scalar.copy(
                    _v(vout, h0 * 640 + j * H, [[640, ng], [80, 8], [1, H]]),
                    tpv[:, :128 * ng])

            def emit_p3(g):
                vout = vouts[g // 2]
                dst = bass.AP(
                    tensor=out_d.tensor,
                    offset=out_d.offset + g * 1024 * 80,
                    ap=[[80, 128], [128 * 80, 8], [1, 80]])
                nc.sync.dma_start(out=dst,
                                  in_=_v(vout, (g % 2) * 640,
                                         [[80, 8], [1, 80]]))

            vouts = [vop.tile([128, 2 * 8 * 80], F16, name=f"vout{k}")
                     for k in range(4)]

            # ---- main schedule: round-robin software-pipelined chains ----
            # Emission order IS the per-engine schedule (strict in-order
            # execution), so: xt DMA dispatches go out as early as the 6
            # rotating buffers allow, P1 matmuls are emitted only where the
            # data has landed by that point of the schedule, the four
            # pair-chains are interleaved, and pairs 0/1 hold back their
            # final iterations as end-game filler while chains 2/3 drain.
            def emit_p1(st, drain=None):
                emit_p1_dma(st)
                emit_p1_mm(st, drain)

            emit_p1(0)
            emit_p1(1)
            emit_uaud()
            emit_blks()
            plan = [
                ("A", 0, 0, 0), ("B", 0, 0, 0), ("p", 2), ("p", 3),
                ("A", 0, 0, 1), ("B", 0, 0, 1),
                ("p", 4), ("p", 5), ("p", 6), ("p", 7),
                ("A", 1, 0, None), ("B", 1, 0, None),
                ("A", 0, 1, None), ("B", 0, 1, None),
                ("p", 8), ("p", 9),
                ("A", 1, 1, None), ("B", 1, 1, None),
                ("p", 10), ("p", 11),
                ("A", 2, 0, None), ("B", 2, 0, None),
                ("p", 12), ("p", 13),
                ("A", 0, 2, None), ("B", 0, 2, None),
                ("p", 14), ("p", 15),
                ("A", 1, 2, None), ("B", 1, 2, None),
                ("A", 3, 0, None), ("B", 3, 0, None),
                ("A", 2, 1, None), ("B", 2, 1, None),
                ("A", 0, 3, None), ("B", 0, 3, None),
                ("A", 1, 3, None), ("B", 1, 3, None),
                ("A", 3, 1, None), ("B", 3, 1, None),
                ("A", 2, 2, None), ("B", 2, 2, None),
                ("A", 0, 4, None), ("B", 0, 4, None),
                ("A", 3, 2, None), ("B", 3, 2, None),
                ("A", 1, 4, None), ("B", 1, 4, None),
                ("A", 2, 3, None), ("B", 2, 3, None),
                ("A", 3, 3, None), ("B", 3, 3, None),
                ("A", 2, 4, None), ("B", 2, 4, None),
                ("A", 3, 4, None), ("B", 3, 4, None),
            ]
            for step in plan:
                if step[0] == "p":
                    emit_p1(step[1])
                elif step[0] == "A":
                    emit_A(step[1], step[2], step[3])
                else:
                    _, q, j, half = step
                    emit_B(q, j, half)
                    if j == SEARCH_NUM - 1:
                        emit_p3(2 * q)
                        emit_p3(2 * q + 1)


def _in_maps(inputs, B_core):
    cst = _host_constants(inputs["W_proj"], inputs["b_proj"],
                          inputs["W_k"], inputs["b_k"])
    x = np.asarray(inputs["item_corpus"], dtype=np.float32)
    u = np.ascontiguousarray(inputs["user_intent"], dtype=np.float32)
    B = x.shape[0]
    NST = B_core // 512
    NG = B_core // 1024
    XTW = NSI * 512 + 128
    # [nst_total, 100, 10, 512]: block (st, d, i, s)
    xb = (x.astype(np.float16)
          .reshape(B // 512, 512, NSI, D)
          .transpose(0, 3, 2, 1))
    # u~_0 = W_k @ user (column form) == user @ W_k.T (row form)
    U0 = (u.astype(np.float64) @ cst["Wk"].T).astype(np.float16)
    maps = []
    for core in range(NCORES):
        xt = np.zeros((NST, 128, XTW), dtype=np.float16)
        xt[:, :D, :NSI * 512] = xb[core * NST:(core + 1) * NST].reshape(
            NST, D, NSI * 512)
        xt[:, D, :NSI * 512] = 1.0
        lo, hi = core * B_core, (core + 1) * B_core
        u0r = U0[lo:hi].reshape(NG, 8, 128, H)
        ua0 = np.ascontiguousarray(
            u0r.transpose(2, 0, 1, 3).reshape(128, NG * 128))
        ud0 = np.ascontiguousarray(
            u0r.transpose(1, 3, 0, 2).reshape(128, NG * 128))
        m = {"xt": xt, "ua0": ua0, "ud0": ud0,
             "waug": cst["waug"], "blks": cst["blks"], "cvdv": cst["cvdv"]}
        maps.append(m)
    return maps


def _assemble(inputs, results):
    u = np.asarray(inputs["user_intent"], dtype=np.float32)
    bs = u.shape[0]
    dev = np.concatenate([np.asarray(r["out"]) for r in results], axis=0)
    out = np.empty((bs, SEARCH_NUM + 1, H), dtype=np.float32)
    out[:, 0, :] = u
    out[:, 1:, :] = dev.astype(np.float32)
    return out


_COMPILED = {}


def _get_nc(B_core):
    if B_core not in _COMPILED:
        nc = bacc.Bacc("TRN2", target_bir_lowering=False, debug=False,
                       num_devices=NCORES)
        build_program(nc, B_core)
        nc.compile()
        _COMPILED[B_core] = nc
    return _COMPILED[B_core]


def kernel(**inputs) -> np.ndarray:
    bs = inputs["user_intent"].shape[0]
    assert bs % NCORES == 0
    B_core = bs // NCORES
    nc = _get_nc(B_core)
    res = run_bass_kernel_spmd(nc, _in_maps(inputs, B_core),
                               core_ids=list(range(NCORES)))
    return _assemble(inputs, res.results)
